# revision 19
# baseline (speedup 1.0000x reference)
"""TRN2 Bass kernel for nn_Attention_24704651887167.

Multi-head attention (B=8, N=1024, C=768, H=12, D=64), f32 in/out.
Data-parallel over batch: one batch element per NeuronCore (8 cores).

Per-core plan:
  A. load x_b [1024,768], transpose on PE -> xT [768,1024] (feature-major)
  B. qkT[f,n] = w_qk @ xT (12 f-tiles, accumulate 6 c-tiles)
     v[n,f]   = x @ w_v.T  (token-major), stored interleaved as v' = [v_h | 1]
                per head (65 columns) for the fused softmax-denominator trick
  C. per head pair: S^T[k,q] = kT_h^T-tiles @ qT_h  (keys on partitions)
     P = exp(S*scale) on ACT (PSUM->SBUF)
     [out_h^T ; denom] = v'_h^T @ P accumulated over key tiles (ones row of v'
     produces the softmax denominator for free); normalize with fast
     reciprocal + gpsimd partition-broadcast; result attnT [768,1024] is
     feature-major -- exactly the lhsT layout the projection needs.
  D. y = attnT^T-tiles @ w_proj^T + bias, token-major, DMA out.

All tensors are allocated per-index (per c-tile / per n-tile) so Tile's
dependency tracking lets phases overlap.  Matmul dtype is fp32r (fp32 data
at ~bf16 PE rate, tf32-like rounding) or bf16 via ATTN_BF16/PROJ_BF16.

Weights are passed transposed from the host (layout prep, done once in numpy).
"""
import os
import sys

for _p in ("/opt/trn_rl_repo", "/opt/pypackages"):
    if _p not in sys.path:
        sys.path.insert(0, _p)

import numpy as np
import concourse.bacc as bacc
import concourse.tile as tile
from concourse import mybir
from concourse.bass_utils import run_bass_kernel_spmd

B, N, C = 8, 1024, 768
H, D = 12, 64
SCALE = D ** -0.5
NT = N // 128       # 8 token tiles
CT = C // 128       # 6 channel tiles
FT_QK = 2 * C // 128  # 12 feature tiles for q|k
F32 = mybir.dt.float32
F32R = mybir.dt.float32r
BF16 = mybir.dt.bfloat16

ATTN_BF16 = os.environ.get("ATTN_BF16", "0") == "1"
A_DT = BF16 if ATTN_BF16 else F32R

_CACHE = {}


def _build():
    nc = bacc.Bacc("TRN2", debug=False, num_devices=B)
    x_d = nc.dram_tensor("x", [N, C], F32, kind="ExternalInput")
    wqkT_d = nc.dram_tensor("wqkT", [C, 2 * C], F32R, kind="ExternalInput")
    wvT_d = nc.dram_tensor("wvT", [C, C], F32R, kind="ExternalInput")
    wpT_d = nc.dram_tensor("wpT", [C, C], F32R, kind="ExternalInput")
    bias_d = nc.dram_tensor("bias", [128, C], F32, kind="ExternalInput")
    y_d = nc.dram_tensor("y", [N, C], F32, kind="ExternalOutput")
    ident_d = nc.inline_tensor(np.eye(128, dtype=np.float32), name="ident")

    with tile.TileContext(nc) as tc:
        with tc.tile_pool(name="persist", bufs=1) as persist:
            ident = persist.tile([128, 128], F32)
            nc.sync.dma_start(ident[:], ident_d[:])

            ones = persist.tile([128, H], F32)
            nc.vector.memset(ones[:], 1.0)
            bias_s = persist.tile([128, C], F32)
            nc.sync.dma_start(bias_s[:], bias_d[:])
            wpT_m = persist.tile([128, CT, C], F32R)
            wpT_s = [wpT_m[:, ct, :] for ct in range(CT)]
            for ct in range(CT):
                nc.sync.dma_start(wpT_s[ct][:], wpT_d[ct * 128:(ct + 1) * 128, :])

            qkT_m = persist.tile([128, FT_QK, N], A_DT)
            vp_m = persist.tile([128, NT, H, D + 1], A_DT)
            attnT_m = persist.tile([128, CT, N], F32R)
            qkT_s = [qkT_m[:, ft, :] for ft in range(FT_QK)]
            vp_s = [vp_m[:, nt, :, :] for nt in range(NT)]
            attnT_s = [attnT_m[:, ct, :] for ct in range(CT)]

            # ---------------- Phase A: transpose x ----------------
            ab_cm = tc.tile_pool(name="ab", bufs=1)
            ab = ab_cm.__enter__()
            xn_m = ab.tile([128, NT, C], F32)
            xn_s = [xn_m[:, nt, :] for nt in range(NT)]
            for nt in range(NT):
                for cc in range(4):
                    nc.sync.dma_start(
                        xn_s[nt][:, cc * 192:(cc + 1) * 192],
                        x_d[nt * 128:(nt + 1) * 128, cc * 192:(cc + 1) * 192],
                    )
            wvT_m = ab.tile([128, CT, C], F32R)
            wvT_s = [wvT_m[:, ct, :] for ct in range(CT)]
            for ct in range(CT):
                nc.sync.dma_start(wvT_s[ct][:], wvT_d[ct * 128:(ct + 1) * 128, :])
            xT_m = ab.tile([128, CT, N], F32R)
            xT_s = [xT_m[:, ct, :] for ct in range(CT)]
            with tc.tile_pool(name="pa", bufs=4, space="PSUM") as pa:
                for ct in range(CT):
                    for nt in range(NT):
                        tp = pa.tile([128, 128], F32, tag="tp")
                        nc.tensor.transpose(
                            tp[:], xn_s[nt][:, ct * 128:(ct + 1) * 128], ident[:]
                        )
                        nc.vector.tensor_copy(
                            xT_s[ct][:, nt * 128:(nt + 1) * 128], tp[:]
                        )

            # ---------------- Phase B: QKV ----------------
            with tc.tile_pool(name="abw", bufs=8) as abw, \
                 tc.tile_pool(name="pb", bufs=2, space="PSUM") as pb:
                # qkT f-tiles in the order phase C consumes them (pairs q/k)
                for ft in [x for j in range(CT) for x in (j, 6 + j)]:
                    qk_p = pb.tile([128, N], F32, tag="qk")
                    for ct in range(CT):
                        w_t = abw.tile([128, 128], F32R, tag="wqk")
                        nc.sync.dma_start(
                            w_t[:],
                            wqkT_d[ct * 128:(ct + 1) * 128,
                                   ft * 128:(ft + 1) * 128],
                        )
                        for qc in range(N // 512):
                            nc.tensor.matmul(
                                qk_p[:, qc * 512:(qc + 1) * 512],
                                w_t[:],
                                xT_s[ct][:, qc * 512:(qc + 1) * 512],
                                start=(ct == 0), stop=(ct == CT - 1),
                            )
                    nc.vector.tensor_copy(qkT_s[ft][:], qk_p[:])

                # v token-major, interleaved into v' = [v_h | 1]
                for nt in range(NT):
                    v_p = pb.tile([128, C], F32, tag="v")
                    for ct in range(CT):
                        for fc, f0, f1 in ((0, 0, 512), (1, 512, C)):
                            nc.tensor.matmul(
                                v_p[:, f0:f1],
                                xT_s[ct][:, nt * 128:(nt + 1) * 128],
                                wvT_s[ct][:, f0:f1],
                                start=(ct == 0), stop=(ct == CT - 1),
                            )
                    nc.vector.tensor_copy(
                        vp_s[nt][:, :, 0:D],
                        v_p[:].rearrange("p (h d) -> p h d", h=H),
                    )
                    nc.vector.tensor_copy(
                        vp_s[nt][:, :, D:D + 1],
                        ones[:].rearrange("p (h o) -> p h o", o=1),
                    )

            ab_cm.__exit__(None, None, None)

            # ------- Phase C: attention, head pairs on disjoint PE rows -------
            with tc.tile_pool(name="cp", bufs=2) as cp, \
                 tc.tile_pool(name="crec", bufs=2) as crec, \
                 tc.tile_pool(name="pse", bufs=1, space="PSUM") as pse_pool, \
                 tc.tile_pool(name="pso", bufs=1, space="PSUM") as pso_pool, \
                 tc.tile_pool(name="po", bufs=1, space="PSUM") as po_pool:
                for j in range(H // 2):
                    qT_t = qkT_s[j]
                    kT_t = qkT_s[6 + j]
                    o_e = po_pool.tile([D + 1, N], F32, tag="oe")
                    o_o = po_pool.tile([D + 1, N], F32, tag="oo")
                    NMM = 512
                    for kt in range(NT):
                        s_e = pse_pool.tile([128, N], F32, tag="se")
                        s_o = pso_pool.tile([128, N], F32, tag="so")
                        p_e = cp.tile([128, N], A_DT, tag="pe")
                        p_o = cp.tile([128, N], A_DT, tag="po")
                        for qc in range(N // NMM):
                            nc.tensor.matmul(
                                s_e[:, qc * NMM:(qc + 1) * NMM],
                                kT_t[0:64, kt * 128:(kt + 1) * 128],
                                qT_t[0:64, qc * NMM:(qc + 1) * NMM],
                                start=True, stop=True,
                            )
                            nc.tensor.matmul(
                                s_o[:, qc * NMM:(qc + 1) * NMM],
                                kT_t[64:128, kt * 128:(kt + 1) * 128],
                                qT_t[64:128, qc * NMM:(qc + 1) * NMM],
                                start=True, stop=True,
                            )
                        nc.scalar.activation(
                            out=p_e[:], in_=s_e[:],
                            func=mybir.ActivationFunctionType.Exp, scale=SCALE,
                        )
                        nc.scalar.activation(
                            out=p_o[:], in_=s_o[:],
                            func=mybir.ActivationFunctionType.Exp, scale=SCALE,
                        )
                        for qc in range(N // NMM):
                            nc.tensor.matmul(
                                o_e[:, qc * NMM:(qc + 1) * NMM],
                                vp_s[kt][:, 2 * j, :],
                                p_e[:, qc * NMM:(qc + 1) * NMM],
                                start=(kt == 0), stop=(kt == NT - 1),
                            )
                            nc.tensor.matmul(
                                o_o[:, qc * NMM:(qc + 1) * NMM],
                                vp_s[kt][:, 2 * j + 1, :],
                                p_o[:, qc * NMM:(qc + 1) * NMM],
                                start=(kt == 0), stop=(kt == NT - 1),
                            )
                    for h, o_p in ((2 * j, o_e), (2 * j + 1, o_o)):
                        r0 = (h % 2) * 64
                        den = crec.tile([1, N], F32, tag="den")
                        nc.vector.tensor_copy(den[:], o_p[D:D + 1, :])
                        denb = crec.tile([64, N], F32, tag="denb")
                        nc.gpsimd.partition_broadcast(denb[:], den[:])
                        recb = crec.tile([64, N], F32, tag="recb")
                        nc.vector.reciprocal_approx_fast(recb[:], denb[:])
                        nc.vector.tensor_mul(
                            attnT_s[j][r0:r0 + 64, :], o_p[0:D, :], recb[:]
                        )

            # ---------------- Phase D: projection + bias ----------------
            with tc.tile_pool(name="dp", bufs=2) as dp, \
                 tc.tile_pool(name="py", bufs=2, space="PSUM") as py_pool:
                for nt in range(NT):
                    y_p = py_pool.tile([128, C], F32, tag="y")
                    for ct in range(CT):
                        for fc, f0, f1 in ((0, 0, 512), (1, 512, C)):
                            nc.tensor.matmul(
                                y_p[:, f0:f1],
                                attnT_s[ct][:, nt * 128:(nt + 1) * 128],
                                wpT_s[ct][:, f0:f1],
                                start=(ct == 0), stop=(ct == CT - 1),
                            )
                    y_s = dp.tile([128, C], F32, tag="ys")
                    nc.vector.tensor_add(y_s[:], y_p[:], bias_s[:])
                    nc.sync.dma_start(y_d[nt * 128:(nt + 1) * 128, :], y_s[:])

    nc.compile()
    return nc


def _get_nc():
    if "nc" not in _CACHE:
        _CACHE["nc"] = _build()
    return _CACHE["nc"]


def _run(x, w_qkv, w_proj, b_proj, trace=False, **kw):
    nc = _get_nc()
    wqkT = np.ascontiguousarray(w_qkv[:2 * C].T.astype(np.float32))
    wvT = np.ascontiguousarray(w_qkv[2 * C:].T.astype(np.float32))
    wpT = np.ascontiguousarray(w_proj.T.astype(np.float32))
    bias = np.ascontiguousarray(
        np.broadcast_to(b_proj.astype(np.float32), (128, C))
    )
    x = np.asarray(x, dtype=np.float32)
    in_maps = [
        {
            "x": np.ascontiguousarray(x[b]),
            "wqkT": wqkT,
            "wvT": wvT,
            "wpT": wpT,
            "bias": bias,
        }
        for b in range(B)
    ]
    out = run_bass_kernel_spmd(nc, in_maps, core_ids=list(range(B)),
                               trace=trace, **kw)
    return out


def kernel(x, w_qkv, w_proj, b_proj):
    res = _run(x, w_qkv, w_proj, b_proj)
    return np.stack([r["y"] for r in res.results], axis=0)


# revision 20
# speedup vs baseline: 1.0166x; 1.0166x over previous
"""TRN2 Bass kernel for nn_Attention_24704651887167.

Multi-head attention (B=8, N=1024, C=768, H=12, D=64), f32 in/out.
Data-parallel over batch: one batch element per NeuronCore (8 cores).

Per-core plan:
  A. load x_b [1024,768], transpose on PE -> xT [768,1024] (feature-major)
  B. qkT[f,n] = w_qk @ xT (12 f-tiles, accumulate 6 c-tiles)
     v[n,f]   = x @ w_v.T  (token-major), stored interleaved as v' = [v_h | 1]
                per head (65 columns) for the fused softmax-denominator trick
  C. per head pair: S^T[k,q] = kT_h^T-tiles @ qT_h  (keys on partitions)
     P = exp(S*scale) on ACT (PSUM->SBUF)
     [out_h^T ; denom] = v'_h^T @ P accumulated over key tiles (ones row of v'
     produces the softmax denominator for free); normalize with fast
     reciprocal + gpsimd partition-broadcast; result attnT [768,1024] is
     feature-major -- exactly the lhsT layout the projection needs.
  D. y = attnT^T-tiles @ w_proj^T + bias, token-major, DMA out.

All tensors are allocated per-index (per c-tile / per n-tile) so Tile's
dependency tracking lets phases overlap.  Matmul dtype is fp32r (fp32 data
at ~bf16 PE rate, tf32-like rounding) or bf16 via ATTN_BF16/PROJ_BF16.

Weights are passed transposed from the host (layout prep, done once in numpy).
"""
import os
import sys

for _p in ("/opt/trn_rl_repo", "/opt/pypackages"):
    if _p not in sys.path:
        sys.path.insert(0, _p)

import numpy as np
import concourse.bacc as bacc
import concourse.tile as tile
from concourse import mybir
from concourse.bass_utils import run_bass_kernel_spmd

B, N, C = 8, 1024, 768
H, D = 12, 64
SCALE = D ** -0.5
NT = N // 128       # 8 token tiles
CT = C // 128       # 6 channel tiles
FT_QK = 2 * C // 128  # 12 feature tiles for q|k
F32 = mybir.dt.float32
F32R = mybir.dt.float32r
BF16 = mybir.dt.bfloat16

ATTN_BF16 = os.environ.get("ATTN_BF16", "0") == "1"
A_DT = BF16 if ATTN_BF16 else F32R

_CACHE = {}


def _build():
    nc = bacc.Bacc("TRN2", debug=False, num_devices=B)
    x_d = nc.dram_tensor("x", [N, C], F32, kind="ExternalInput")
    wqkT_d = nc.dram_tensor("wqkT", [C, 2 * C], F32R, kind="ExternalInput")
    wvT_d = nc.dram_tensor("wvT", [C, C], F32R, kind="ExternalInput")
    wpT_d = nc.dram_tensor("wpT", [C, C], F32R, kind="ExternalInput")
    bias_d = nc.dram_tensor("bias", [128, C], F32, kind="ExternalInput")
    y_d = nc.dram_tensor("y", [N, C], F32, kind="ExternalOutput")
    ident_d = nc.inline_tensor(np.eye(128, dtype=np.float32), name="ident")

    with tile.TileContext(nc) as tc:
        with tc.tile_pool(name="persist", bufs=1) as persist:
            ident = persist.tile([128, 128], F32)
            nc.sync.dma_start(ident[:], ident_d[:])

            ones = persist.tile([128, H], F32)
            nc.vector.memset(ones[:], 1.0)
            bias_s = persist.tile([128, C], F32)
            nc.sync.dma_start(bias_s[:], bias_d[:])
            wpT_s = [persist.tile([128, C], F32R, tag=f"wp{ct}", name=f"wp{ct}") for ct in range(CT)]
            for ct in range(CT):
                nc.sync.dma_start(wpT_s[ct][:], wpT_d[ct * 128:(ct + 1) * 128, :])

            qkT_s = [persist.tile([128, N], A_DT, tag=f"qkT{ft}", name=f"qkT{ft}")
                     for ft in range(FT_QK)]
            vp_s = [persist.tile([128, H, D + 1], A_DT, tag=f"vp{nt}", name=f"vp{nt}")
                    for nt in range(NT)]
            attnT_s = [persist.tile([128, N], F32R, tag=f"at{ct}", name=f"at{ct}")
                       for ct in range(CT)]

            # ---------------- Phase A: transpose x ----------------
            ab_cm = tc.tile_pool(name="ab", bufs=1)
            ab = ab_cm.__enter__()
            xn_s = [ab.tile([128, C], F32, tag=f"xn{nt}", name=f"xn{nt}")
                    for nt in range(NT)]
            for nt in range(NT):
                for cc in range(4):
                    nc.sync.dma_start(
                        xn_s[nt][:, cc * 192:(cc + 1) * 192],
                        x_d[nt * 128:(nt + 1) * 128, cc * 192:(cc + 1) * 192],
                    )
            wvT_s = [ab.tile([128, C], F32R, tag=f"wv{ct}", name=f"wv{ct}")
                     for ct in range(CT)]
            for ct in range(CT):
                nc.sync.dma_start(wvT_s[ct][:], wvT_d[ct * 128:(ct + 1) * 128, :])
            xT_s = [ab.tile([128, N], F32R, tag=f"xT{ct}", name=f"xT{ct}")
                    for ct in range(CT)]
            with tc.tile_pool(name="pa", bufs=6, space="PSUM") as pa:
                for ct in range(CT):
                    for nt in range(NT):
                        tp = pa.tile([128, 128], F32, tag="tp")
                        nc.tensor.transpose(
                            tp[:], xn_s[nt][:, ct * 128:(ct + 1) * 128], ident[:]
                        )
                        nc.vector.tensor_copy(
                            xT_s[ct][:, nt * 128:(nt + 1) * 128], tp[:]
                        )

            # ---------------- Phase B: QKV ----------------
            with tc.tile_pool(name="abw", bufs=8) as abw, \
                 tc.tile_pool(name="pb", bufs=2, space="PSUM") as pb:
                # qkT f-tiles in the order phase C consumes them (pairs q/k)
                for ft in [x for j in range(CT) for x in (j, 6 + j)]:
                    qk_p = pb.tile([128, N], F32, tag="qk")
                    for ct in range(CT):
                        w_t = abw.tile([128, 128], F32R, tag="wqk")
                        nc.sync.dma_start(
                            w_t[:],
                            wqkT_d[ct * 128:(ct + 1) * 128,
                                   ft * 128:(ft + 1) * 128],
                        )
                        for qc in range(N // 512):
                            nc.tensor.matmul(
                                qk_p[:, qc * 512:(qc + 1) * 512],
                                w_t[:],
                                xT_s[ct][:, qc * 512:(qc + 1) * 512],
                                start=(ct == 0), stop=(ct == CT - 1),
                            )
                    nc.vector.tensor_copy(qkT_s[ft][:], qk_p[:])

                # v token-major, interleaved into v' = [v_h | 1]
                for nt in range(NT):
                    v_p = pb.tile([128, C], F32, tag="v")
                    for ct in range(CT):
                        for fc, f0, f1 in ((0, 0, 512), (1, 512, C)):
                            nc.tensor.matmul(
                                v_p[:, f0:f1],
                                xT_s[ct][:, nt * 128:(nt + 1) * 128],
                                wvT_s[ct][:, f0:f1],
                                start=(ct == 0), stop=(ct == CT - 1),
                            )
                    nc.vector.tensor_copy(
                        vp_s[nt][:, :, 0:D],
                        v_p[:].rearrange("p (h d) -> p h d", h=H),
                    )
                    nc.vector.tensor_copy(
                        vp_s[nt][:, :, D:D + 1],
                        ones[:].rearrange("p (h o) -> p h o", o=1),
                    )

            ab_cm.__exit__(None, None, None)

            # ------- Phase C: attention, head pairs on disjoint PE rows -------
            with tc.tile_pool(name="cp", bufs=2) as cp, \
                 tc.tile_pool(name="crec", bufs=2) as crec, \
                 tc.tile_pool(name="pse", bufs=1, space="PSUM") as pse_pool, \
                 tc.tile_pool(name="pso", bufs=1, space="PSUM") as pso_pool, \
                 tc.tile_pool(name="po", bufs=1, space="PSUM") as po_pool:
                for j in range(H // 2):
                    qT_t = qkT_s[j]
                    kT_t = qkT_s[6 + j]
                    o_e = po_pool.tile([D + 1, N], F32, tag="oe")
                    o_o = po_pool.tile([D + 1, N], F32, tag="oo")
                    NMM = 512
                    for kt in range(NT):
                        s_e = pse_pool.tile([128, N], F32, tag="se")
                        s_o = pso_pool.tile([128, N], F32, tag="so")
                        p_e = cp.tile([128, N], A_DT, tag="pe")
                        p_o = cp.tile([128, N], A_DT, tag="po")
                        for qc in range(N // NMM):
                            nc.tensor.matmul(
                                s_e[:, qc * NMM:(qc + 1) * NMM],
                                kT_t[0:64, kt * 128:(kt + 1) * 128],
                                qT_t[0:64, qc * NMM:(qc + 1) * NMM],
                                start=True, stop=True,
                            )
                            nc.tensor.matmul(
                                s_o[:, qc * NMM:(qc + 1) * NMM],
                                kT_t[64:128, kt * 128:(kt + 1) * 128],
                                qT_t[64:128, qc * NMM:(qc + 1) * NMM],
                                start=True, stop=True,
                            )
                        nc.scalar.activation(
                            out=p_e[:], in_=s_e[:],
                            func=mybir.ActivationFunctionType.Exp, scale=SCALE,
                        )
                        nc.scalar.activation(
                            out=p_o[:], in_=s_o[:],
                            func=mybir.ActivationFunctionType.Exp, scale=SCALE,
                        )
                        for qc in range(N // NMM):
                            nc.tensor.matmul(
                                o_e[:, qc * NMM:(qc + 1) * NMM],
                                vp_s[kt][:, 2 * j, :],
                                p_e[:, qc * NMM:(qc + 1) * NMM],
                                start=(kt == 0), stop=(kt == NT - 1),
                            )
                            nc.tensor.matmul(
                                o_o[:, qc * NMM:(qc + 1) * NMM],
                                vp_s[kt][:, 2 * j + 1, :],
                                p_o[:, qc * NMM:(qc + 1) * NMM],
                                start=(kt == 0), stop=(kt == NT - 1),
                            )
                    for h, o_p in ((2 * j, o_e), (2 * j + 1, o_o)):
                        r0 = (h % 2) * 64
                        den = crec.tile([1, N], F32, tag="den")
                        nc.vector.tensor_copy(den[:], o_p[D:D + 1, :])
                        denb = crec.tile([64, N], F32, tag="denb")
                        nc.gpsimd.partition_broadcast(denb[:], den[:])
                        recb = crec.tile([64, N], F32, tag="recb")
                        nc.vector.reciprocal_approx_fast(recb[:], denb[:])
                        nc.vector.tensor_mul(
                            attnT_s[j][r0:r0 + 64, :], o_p[0:D, :], recb[:]
                        )

            # ---------------- Phase D: projection + bias ----------------
            with tc.tile_pool(name="dp", bufs=2) as dp, \
                 tc.tile_pool(name="py", bufs=2, space="PSUM") as py_pool:
                for nt in range(NT):
                    y_p = py_pool.tile([128, C], F32, tag="y")
                    for ct in range(CT):
                        for fc, f0, f1 in ((0, 0, 512), (1, 512, C)):
                            nc.tensor.matmul(
                                y_p[:, f0:f1],
                                attnT_s[ct][:, nt * 128:(nt + 1) * 128],
                                wpT_s[ct][:, f0:f1],
                                start=(ct == 0), stop=(ct == CT - 1),
                            )
                    y_s = dp.tile([128, C], F32, tag="ys")
                    nc.vector.tensor_add(y_s[:], y_p[:], bias_s[:])
                    nc.sync.dma_start(y_d[nt * 128:(nt + 1) * 128, :], y_s[:])

    nc.compile()
    return nc


def _get_nc():
    if "nc" not in _CACHE:
        _CACHE["nc"] = _build()
    return _CACHE["nc"]


def _run(x, w_qkv, w_proj, b_proj, trace=False, **kw):
    nc = _get_nc()
    wqkT = np.ascontiguousarray(w_qkv[:2 * C].T.astype(np.float32))
    wvT = np.ascontiguousarray(w_qkv[2 * C:].T.astype(np.float32))
    wpT = np.ascontiguousarray(w_proj.T.astype(np.float32))
    bias = np.ascontiguousarray(
        np.broadcast_to(b_proj.astype(np.float32), (128, C))
    )
    x = np.asarray(x, dtype=np.float32)
    in_maps = [
        {
            "x": np.ascontiguousarray(x[b]),
            "wqkT": wqkT,
            "wvT": wvT,
            "wpT": wpT,
            "bias": bias,
        }
        for b in range(B)
    ]
    out = run_bass_kernel_spmd(nc, in_maps, core_ids=list(range(B)),
                               trace=trace, **kw)
    return out


def kernel(x, w_qkv, w_proj, b_proj):
    res = _run(x, w_qkv, w_proj, b_proj)
    return np.stack([r["y"] for r in res.results], axis=0)


# revision 21
# speedup vs baseline: 1.0752x; 1.0576x over previous
"""TRN2 Bass kernel for nn_Attention_24704651887167.

Multi-head attention (B=8, N=1024, C=768, H=12, D=64), f32 in/out.
Data-parallel over batch: one batch element per NeuronCore (8 cores).

Per-core plan:
  A. load x_b [1024,768], transpose on PE -> xT [768,1024] (feature-major)
  B. qkT[f,n] = w_qk @ xT (12 f-tiles, accumulate 6 c-tiles)
     v[n,f]   = x @ w_v.T  (token-major), stored interleaved as v' = [v_h | 1]
                per head (65 columns) for the fused softmax-denominator trick
  C. per head pair: S^T[k,q] = kT_h^T-tiles @ qT_h  (keys on partitions)
     P = exp(S*scale) on ACT (PSUM->SBUF)
     [out_h^T ; denom] = v'_h^T @ P accumulated over key tiles (ones row of v'
     produces the softmax denominator for free); normalize with fast
     reciprocal + gpsimd partition-broadcast; result attnT [768,1024] is
     feature-major -- exactly the lhsT layout the projection needs.
  D. y = attnT^T-tiles @ w_proj^T + bias, token-major, DMA out.

All tensors are allocated per-index (per c-tile / per n-tile) so Tile's
dependency tracking lets phases overlap.  Matmul dtype is fp32r (fp32 data
at ~bf16 PE rate, tf32-like rounding) or bf16 via ATTN_BF16/PROJ_BF16.

Weights are passed transposed from the host (layout prep, done once in numpy).
"""
import os
import sys

for _p in ("/opt/trn_rl_repo", "/opt/pypackages"):
    if _p not in sys.path:
        sys.path.insert(0, _p)

import numpy as np
import concourse.bacc as bacc
import concourse.tile as tile
from concourse import mybir
from concourse.bass_utils import run_bass_kernel_spmd

B, N, C = 8, 1024, 768
H, D = 12, 64
SCALE = D ** -0.5
NT = N // 128       # 8 token tiles
CT = C // 128       # 6 channel tiles
FT_QK = 2 * C // 128  # 12 feature tiles for q|k
F32 = mybir.dt.float32
F32R = mybir.dt.float32r
BF16 = mybir.dt.bfloat16

ATTN_BF16 = os.environ.get("ATTN_BF16", "0") == "1"
A_DT = BF16 if ATTN_BF16 else F32R

_CACHE = {}


def _build():
    nc = bacc.Bacc("TRN2", debug=False, num_devices=B)
    x_d = nc.dram_tensor("x", [N, C], F32, kind="ExternalInput")
    wqkT_d = nc.dram_tensor("wqkT", [C, 2 * C], F32R, kind="ExternalInput")
    wvT_d = nc.dram_tensor("wvT", [C, C], F32R, kind="ExternalInput")
    wpT_d = nc.dram_tensor("wpT", [C, C], F32R, kind="ExternalInput")
    bias_d = nc.dram_tensor("bias", [128, C], F32, kind="ExternalInput")
    y_d = nc.dram_tensor("y", [N, C], F32, kind="ExternalOutput")
    ident_d = nc.inline_tensor(np.eye(128, dtype=np.float32), name="ident")

    with tile.TileContext(nc) as tc:
        with tc.tile_pool(name="persist", bufs=1) as persist:
            ident = persist.tile([128, 128], F32)
            nc.sync.dma_start(ident[:], ident_d[:])

            ones = persist.tile([128, H], F32)
            nc.vector.memset(ones[:], 1.0)
            bias_s = persist.tile([128, C], F32)
            nc.sync.dma_start(bias_s[:], bias_d[:])
            wpT_s = [persist.tile([128, C], F32R, tag=f"wp{ct}", name=f"wp{ct}") for ct in range(CT)]
            for ct in range(CT):
                nc.sync.dma_start(wpT_s[ct][:], wpT_d[ct * 128:(ct + 1) * 128, :])

            qkT_s = [persist.tile([128, N], A_DT, tag=f"qkT{ft}", name=f"qkT{ft}")
                     for ft in range(FT_QK)]
            vp_s = [persist.tile([128, H, D + 1], A_DT, tag=f"vp{nt}", name=f"vp{nt}")
                    for nt in range(NT)]
            attnT_s = [persist.tile([128, N], F32R, tag=f"at{ct}", name=f"at{ct}")
                       for ct in range(CT)]

            # ---------------- Phase A: transpose x ----------------
            ab_cm = tc.tile_pool(name="ab", bufs=1)
            ab = ab_cm.__enter__()
            xn_s = [ab.tile([128, C], F32, tag=f"xn{nt}", name=f"xn{nt}")
                    for nt in range(NT)]
            for nt in range(NT):
                for cc in range(4):
                    nc.sync.dma_start(
                        xn_s[nt][:, cc * 192:(cc + 1) * 192],
                        x_d[nt * 128:(nt + 1) * 128, cc * 192:(cc + 1) * 192],
                    )
            wvT_s = [ab.tile([128, C], F32R, tag=f"wv{ct}", name=f"wv{ct}")
                     for ct in range(CT)]
            for ct in range(CT):
                nc.sync.dma_start(wvT_s[ct][:], wvT_d[ct * 128:(ct + 1) * 128, :])
            xT_s = [ab.tile([128, N], F32R, tag=f"xT{ct}", name=f"xT{ct}")
                    for ct in range(CT)]
            with tc.tile_pool(name="pa", bufs=6, space="PSUM") as pa:
                for ct in range(CT):
                    for nt in range(NT):
                        tp = pa.tile([128, 128], F32, tag="tp")
                        nc.tensor.transpose(
                            tp[:], xn_s[nt][:, ct * 128:(ct + 1) * 128], ident[:]
                        )
                        nc.vector.tensor_copy(
                            xT_s[ct][:, nt * 128:(nt + 1) * 128], tp[:]
                        )

            # ---------------- Phase B: QKV ----------------
            with tc.tile_pool(name="abw", bufs=8) as abw, \
                 tc.tile_pool(name="pb", bufs=2, space="PSUM") as pb:
                # qkT f-tiles in the order phase C consumes them (pairs q/k)
                for ft in [x for j in range(CT) for x in (j, 6 + j)]:
                    qk_p = pb.tile([128, N], F32, tag="qk")
                    for ct in range(CT):
                        w_t = abw.tile([128, 128], F32R, tag="wqk")
                        nc.sync.dma_start(
                            w_t[:],
                            wqkT_d[ct * 128:(ct + 1) * 128,
                                   ft * 128:(ft + 1) * 128],
                        )
                        for qc in range(N // 512):
                            nc.tensor.matmul(
                                qk_p[:, qc * 512:(qc + 1) * 512],
                                w_t[:],
                                xT_s[ct][:, qc * 512:(qc + 1) * 512],
                                start=(ct == 0), stop=(ct == CT - 1),
                            )
                    nc.vector.tensor_copy(qkT_s[ft][:], qk_p[:])

                # v token-major, interleaved into v' = [v_h | 1]
                for nt in range(NT):
                    v_p = pb.tile([128, C], F32, tag="v")
                    for ct in range(CT):
                        for fc, f0, f1 in ((0, 0, 512), (1, 512, C)):
                            nc.tensor.matmul(
                                v_p[:, f0:f1],
                                xT_s[ct][:, nt * 128:(nt + 1) * 128],
                                wvT_s[ct][:, f0:f1],
                                start=(ct == 0), stop=(ct == CT - 1),
                            )
                    nc.vector.tensor_copy(
                        vp_s[nt][:, :, 0:D],
                        v_p[:].rearrange("p (h d) -> p h d", h=H),
                    )
                    nc.vector.tensor_copy(
                        vp_s[nt][:, :, D:D + 1],
                        ones[:].rearrange("p (h o) -> p h o", o=1),
                    )

            ab_cm.__exit__(None, None, None)

            # ------- Phase C: attention, head pairs on disjoint PE rows -------
            with tc.tile_pool(name="cp", bufs=3) as cp, \
                 tc.tile_pool(name="crec", bufs=2) as crec, \
                 tc.tile_pool(name="pse", bufs=1, space="PSUM") as pse_pool, \
                 tc.tile_pool(name="pso", bufs=1, space="PSUM") as pso_pool, \
                 tc.tile_pool(name="po", bufs=1, space="PSUM") as po_pool:
                for j in range(H // 2):
                    qT_t = qkT_s[j]
                    kT_t = qkT_s[6 + j]
                    o_e = po_pool.tile([D + 1, N], F32, tag="oe")
                    o_o = po_pool.tile([D + 1, N], F32, tag="oo")
                    NMM = 512
                    for kt in range(NT):
                        s_e = pse_pool.tile([128, N], F32, tag="se")
                        s_o = pso_pool.tile([128, N], F32, tag="so")
                        p_e = cp.tile([128, N], A_DT, tag="pe")
                        p_o = cp.tile([128, N], A_DT, tag="po")
                        for qc in range(N // NMM):
                            nc.tensor.matmul(
                                s_e[:, qc * NMM:(qc + 1) * NMM],
                                kT_t[0:64, kt * 128:(kt + 1) * 128],
                                qT_t[0:64, qc * NMM:(qc + 1) * NMM],
                                start=True, stop=True,
                            )
                            nc.tensor.matmul(
                                s_o[:, qc * NMM:(qc + 1) * NMM],
                                kT_t[64:128, kt * 128:(kt + 1) * 128],
                                qT_t[64:128, qc * NMM:(qc + 1) * NMM],
                                start=True, stop=True,
                            )
                        nc.scalar.activation(
                            out=p_e[:], in_=s_e[:],
                            func=mybir.ActivationFunctionType.Exp, scale=SCALE,
                        )
                        nc.scalar.activation(
                            out=p_o[:], in_=s_o[:],
                            func=mybir.ActivationFunctionType.Exp, scale=SCALE,
                        )
                        for qc in range(N // NMM):
                            nc.tensor.matmul(
                                o_e[:, qc * NMM:(qc + 1) * NMM],
                                vp_s[kt][:, 2 * j, :],
                                p_e[:, qc * NMM:(qc + 1) * NMM],
                                start=(kt == 0), stop=(kt == NT - 1),
                            )
                            nc.tensor.matmul(
                                o_o[:, qc * NMM:(qc + 1) * NMM],
                                vp_s[kt][:, 2 * j + 1, :],
                                p_o[:, qc * NMM:(qc + 1) * NMM],
                                start=(kt == 0), stop=(kt == NT - 1),
                            )
                    for h, o_p in ((2 * j, o_e), (2 * j + 1, o_o)):
                        r0 = (h % 2) * 64
                        den = crec.tile([1, N], F32, tag="den")
                        nc.vector.tensor_copy(den[:], o_p[D:D + 1, :])
                        denb = crec.tile([64, N], F32, tag="denb")
                        nc.gpsimd.partition_broadcast(denb[:], den[:])
                        recb = crec.tile([64, N], F32, tag="recb")
                        nc.vector.reciprocal_approx_fast(recb[:], denb[:])
                        nc.vector.tensor_mul(
                            attnT_s[j][r0:r0 + 64, :], o_p[0:D, :], recb[:]
                        )

            # ---------------- Phase D: projection + bias ----------------
            with tc.tile_pool(name="dp", bufs=4) as dp, \
                 tc.tile_pool(name="py", bufs=3, space="PSUM") as py_pool:
                for nt in range(NT):
                    y_p = py_pool.tile([128, C], F32, tag="y")
                    for ct in range(CT):
                        for fc, f0, f1 in ((0, 0, 512), (1, 512, C)):
                            nc.tensor.matmul(
                                y_p[:, f0:f1],
                                attnT_s[ct][:, nt * 128:(nt + 1) * 128],
                                wpT_s[ct][:, f0:f1],
                                start=(ct == 0), stop=(ct == CT - 1),
                            )
                    y_s = dp.tile([128, C], F32, tag="ys")
                    nc.vector.tensor_add(y_s[:], y_p[:], bias_s[:])
                    nc.sync.dma_start(y_d[nt * 128:(nt + 1) * 128, :], y_s[:])

    nc.compile()
    return nc


def _get_nc():
    if "nc" not in _CACHE:
        _CACHE["nc"] = _build()
    return _CACHE["nc"]


def _run(x, w_qkv, w_proj, b_proj, trace=False, **kw):
    nc = _get_nc()
    wqkT = np.ascontiguousarray(w_qkv[:2 * C].T.astype(np.float32))
    wvT = np.ascontiguousarray(w_qkv[2 * C:].T.astype(np.float32))
    wpT = np.ascontiguousarray(w_proj.T.astype(np.float32))
    bias = np.ascontiguousarray(
        np.broadcast_to(b_proj.astype(np.float32), (128, C))
    )
    x = np.asarray(x, dtype=np.float32)
    in_maps = [
        {
            "x": np.ascontiguousarray(x[b]),
            "wqkT": wqkT,
            "wvT": wvT,
            "wpT": wpT,
            "bias": bias,
        }
        for b in range(B)
    ]
    out = run_bass_kernel_spmd(nc, in_maps, core_ids=list(range(B)),
                               trace=trace, **kw)
    return out


def kernel(x, w_qkv, w_proj, b_proj):
    res = _run(x, w_qkv, w_proj, b_proj)
    return np.stack([r["y"] for r in res.results], axis=0)
